# revision 8
# baseline (speedup 1.0000x reference)
"""Bilateral filter (5x5, sigma_r=0.1) on 8 trn2 cores — V2.

Data parallel: (4,3,512,512) reflect-padded, cut into 1024 blocks of 32x32
(36x36 with halo, 3 channels planar); 128 blocks per core = one SBUF
partition each.

Engine plan (per pair d of the 12 symmetric offsets, all 3 channels in one
op):
    d  = xbe[n+delta] - xbe[n]     bf16 (DVE 2x / gpsimd, knob per pair)
    q  = d*d (in place)            bf16 (DVE 2x / gpsimd / ACT Square)
    w  = exp(-alpha2*q + ln sk)    ACT, one op per pair (3ch merged)
    mm = w*x', uu = w*x            bf16 DVE 2x
    PSUM[wx] += mm + uu(shifted); PSUM[w] += w_center + w_shift  (PE bf16)
plus ones/x_center streams folding +1 and the center tap into PSUM, then
out = PSUM[wx] * recip(PSUM[w]).  All accumulation f32 in PSUM.
"""

import sys

for _p in ("/opt/trn_rl_repo",):
    if _p not in sys.path:
        sys.path.insert(0, _p)

import math
import numpy as np
from numpy.lib.stride_tricks import as_strided

KS = 5
PAD = KS // 2
SIGMA_RANGE = 0.1
B, C, H, W = 4, 3, 512, 512
BLK = 32
SB = BLK + 2 * PAD  # 36
NCORES = 8
NBH = H // BLK
NBW = W // BLK
UNITS = B * NBH * NBW  # 1024
UPC = UNITS // NCORES  # 128
GRID = SB * SB  # 1296
GN = C * GRID  # 3888

ALPHA2 = 1.0 / (2.0 * SIGMA_RANGE ** 2)  # 50

PAIRS = [
    (a, b)
    for a in range(0, PAD + 1)
    for b in range(-PAD, PAD + 1)
    if (a > 0) or (a == 0 and b > 0)
]

# engine knobs: per-pair engine for the subtract and the square
# 'dve' | 'pool' | 'act' (act only for squares)
SUB_ENG = ["dve", "pool", "dve", "pool", "dve", "dve",
           "pool", "dve", "dve", "pool", "dve", "dve"]
SQ_ENG = ["act", "act", "dve", "act", "act", "dve",
          "act", "act", "dve", "act", "act", "dve"]
CH_PASSES = [(0, 1), (1, 2), (2, 3)]  # [c0, c1) channel ranges per psum pass

TRACE = False
LAST_STATS = {}
_cache = {}


def _build(sk_flat, repeat=1):
    import concourse.bacc as bacc
    import concourse.tile as tile
    from concourse import mybir
    from contextlib import ExitStack

    f32 = mybir.dt.float32
    bf16 = mybir.dt.bfloat16
    nc = bacc.Bacc(None)
    xbe_h = nc.dram_tensor("xbe", [UPC, GN], bf16, kind="ExternalInput")
    xbo_h = nc.dram_tensor("xbo", [UPC, GN], bf16, kind="ExternalInput")
    out_h = nc.dram_tensor("out", [UPC, C * BLK * BLK], f32, kind="ExternalOutput")
    ident_h = nc.inline_tensor(np.eye(UPC, dtype=np.float32), "ident")

    with tile.TileContext(nc) as tc, ExitStack() as ctx:
        xin = ctx.enter_context(tc.tile_pool(name="xin", bufs=1))
        consts = ctx.enter_context(tc.tile_pool(name="consts", bufs=1))
        work = ctx.enter_context(tc.tile_pool(name="work", bufs=10))
        ep = ctx.enter_context(tc.tile_pool(name="ep", bufs=2))
        psum = ctx.enter_context(tc.tile_pool(name="psum", bufs=2, space="PSUM"))

        identf = consts.tile([UPC, UPC], f32, tag="identf", name="identf")
        nc.sync.dma_start(out=identf[:], in_=ident_h[:])
        identb = consts.tile([UPC, UPC], bf16, tag="identb", name="identb")
        nc.vector.tensor_copy(identb[:], identf[:])
        ones512 = consts.tile([UPC, 512], bf16, tag="ones", name="ones512")
        nc.vector.memset(ones512[:], 1.0)

        lns_map = {}
        bias_tiles = {}
        for (a, b) in PAIRS:
            v = round(float(np.log(sk_flat[(a + PAD) * KS + (b + PAD)])), 9)
            lns_map[(a, b)] = v
            if v not in bias_tiles:
                bt = consts.tile([UPC, 1], f32, tag=f"lns{v}",
                                 name=f"lns{len(bias_tiles)}")
                nc.vector.memset(bt[:], v)
                bias_tiles[v] = bt

        xbe = xin.tile([UPC, C, SB, SB], bf16, name="xbe")
        nc.sync.dma_start(out=xbe[:].rearrange("p a b c -> p (a b c)"), in_=xbe_h[:])
        xbo = xin.tile([UPC, C, SB, SB], bf16, name="xbo")
        nc.sync.dma_start(out=xbo[:].rearrange("p a b c -> p (a b c)"), in_=xbo_h[:])

        def xb_at(rr0, rr1, cb0, width, ch=slice(None)):
            """bf16 x view with even column start (4B aligned) for real HW."""
            if cb0 % 2 == 0:
                return xbe[:, ch, rr0:rr1, cb0:cb0 + width]
            return xbo[:, ch, rr0:rr1, cb0 - 1:cb0 - 1 + width]

        for _rep in range(repeat):
            o_full = ep.tile([UPC, C, BLK, BLK], f32, tag="o_full", name=f"of{_rep}")
            hb = BLK // 2

            for pno, (pc0, pc1) in enumerate(CH_PASSES):
                nch = pc1 - pc0
                chs = slice(pc0, pc1)
                pwx = psum.tile([UPC, nch * BLK * BLK], f32, tag="pwx",
                                name=f"pwx{_rep}_{pno}")
                pw = psum.tile([UPC, nch * BLK * BLK], f32, tag="pw",
                               name=f"pw{_rep}_{pno}")

                for pi, (a, b) in enumerate(PAIRS):
                    r0, r1 = PAD - a, PAD + BLK
                    sr = PAD - a
                    sc = PAD - b
                    uo = sc % 2  # uu is widened to an even column start
                    uw = BLK + 2 * uo
                    # region covers the union of center window, shifted
                    # window, and the widened uu read
                    c0 = min(PAD - max(0, b), sc - uo)
                    c1 = max(PAD + BLK - min(0, b), sc - uo + uw)

                    dq = work.tile([UPC, nch, SB, SB], bf16, tag="dq",
                                   name=f"d{_rep}_{pno}_{pi}")
                    dv = dq[:, :, r0:r1, c0:c1]
                    in_shift = xb_at(r0 + a, r1 + a, c0 + b, c1 - c0, chs)
                    in_base = xbe[:, chs, r0:r1, c0:c1]
                    seng = nc.gpsimd if SUB_ENG[pi] == "pool" else nc.vector
                    seng.tensor_sub(dv, in_shift, in_base)

                    sq = SQ_ENG[pi]
                    if sq == "act":
                        nc.scalar.activation(dv, dv,
                                             mybir.ActivationFunctionType.Square)
                    else:
                        qeng = nc.gpsimd if sq == "pool" else nc.vector
                        qeng.tensor_mul(dv, dv, dv)

                    w = work.tile([UPC, nch, SB, SB], bf16, tag="w",
                                  name=f"w{_rep}_{pno}_{pi}")
                    nc.scalar.activation(
                        w[:, :, r0:r1, c0:c1], dv,
                        mybir.ActivationFunctionType.Exp,
                        bias=bias_tiles[lns_map[(a, b)]][:], scale=-ALPHA2,
                    )

                    mm = work.tile([UPC, nch, BLK, BLK], bf16, tag="mm",
                                   name=f"mm{_rep}_{pno}_{pi}")
                    nc.vector.tensor_mul(
                        mm[:], w[:, :, PAD:PAD + BLK, PAD:PAD + BLK],
                        xb_at(PAD + a, PAD + a + BLK, PAD + b, BLK, chs),
                    )
                    uu = work.tile([UPC, nch, BLK, BLK + 2], bf16, tag="uu",
                                   name=f"uu{_rep}_{pno}_{pi}")
                    nc.vector.tensor_mul(
                        uu[:, :, :, :uw],
                        w[:, :, sr:sr + BLK, sc - uo:sc - uo + uw],
                        xb_at(sr, sr + BLK, sc - uo, uw, chs),
                    )

                    first = pi == 0
                    for ci in range(nch):
                        for h in range(2):
                            rows = slice(h * hb, (h + 1) * hb)
                            cols = slice(ci * 1024 + h * 512,
                                         ci * 1024 + (h + 1) * 512)
                            nc.tensor.matmul(
                                pwx[:, cols], identb[:], mm[:, ci, rows],
                                start=first, stop=False,
                            )
                            nc.tensor.matmul(
                                pwx[:, cols], identb[:],
                                uu[:, ci, rows, uo:uo + BLK],
                                start=False, stop=False,
                            )
                            nc.tensor.matmul(
                                pw[:, cols], identb[:],
                                w[:, ci, PAD + h * hb:PAD + (h + 1) * hb,
                                  PAD:PAD + BLK],
                                start=first, stop=False,
                            )
                            nc.tensor.matmul(
                                pw[:, cols], identb[:],
                                w[:, ci, sr + h * hb:sr + (h + 1) * hb,
                                  sc:sc + BLK],
                                start=False, stop=False,
                            )

                # close groups: + x_center into pwx, + 1 into pw
                for ci in range(nch):
                    for h in range(2):
                        rows = slice(PAD + h * hb, PAD + (h + 1) * hb)
                        cols = slice(ci * 1024 + h * 512,
                                     ci * 1024 + (h + 1) * 512)
                        nc.tensor.matmul(
                            pwx[:, cols], identb[:],
                            xbe[:, pc0 + ci, rows, PAD:PAD + BLK],
                            start=False, stop=True,
                        )
                        nc.tensor.matmul(
                            pw[:, cols], identb[:], ones512[:],
                            start=False, stop=True,
                        )

                # epilogue: out = pwx * recip(pw)
                for ci in range(nch):
                    cols = slice(ci * 1024, (ci + 1) * 1024)
                    rr = ep.tile([UPC, BLK * BLK], f32, tag="rr",
                                 name=f"rr{_rep}_{pno}_{ci}")
                    nc.vector.reciprocal_approx_fast(rr[:], pw[:, cols])
                    nc.vector.tensor_mul(
                        o_full[:, pc0 + ci].rearrange("p a b -> p (a b)"),
                        pwx[:, cols], rr[:],
                    )
            nc.sync.dma_start(
                out=out_h[:], in_=o_full[:].rearrange("p a b c -> p (a b c)")
            )
    nc.finalize()
    return nc


def _shard(x):
    xp = np.pad(x, ((0, 0), (0, 0), (PAD, PAD), (PAD, PAD)), mode="reflect")
    xp = np.ascontiguousarray(xp)
    sb, sc, sh, sw = xp.strides
    v = as_strided(
        xp,
        shape=(B, NBH, NBW, C, SB, SB),
        strides=(sb, BLK * sh, BLK * sw, sc, sh, sw),
    )
    return np.ascontiguousarray(v).reshape(NCORES, UPC, GN)


def _unshard(outs):
    o = outs.reshape(B, NBH, NBW, C, BLK, BLK)
    return np.ascontiguousarray(o.transpose(0, 3, 1, 4, 2, 5).reshape(B, C, H, W))


def _inputs_for(x):
    import ml_dtypes

    shards = _shard(x)  # (8, 128, GN) f32
    xbe = shards.astype(ml_dtypes.bfloat16)
    xbo = np.empty_like(xbe)
    xbo[:, :, :-1] = xbe[:, :, 1:]
    xbo[:, :, -1] = 0
    return xbe, xbo


def _pjrt_parts(nc):
    from concourse import bass2jax, mybir
    import jax

    bass2jax.install_neuronx_cc_hook()
    partition_name = nc.partition_id_tensor.name if nc.partition_id_tensor else None
    in_names, out_names, out_avals, zero_outs = [], [], [], []
    for alloc in nc.m.functions[0].allocations:
        if not isinstance(alloc, mybir.MemoryLocationSet):
            continue
        name = alloc.memorylocations[0].name
        if alloc.kind == "ExternalInput":
            if name != partition_name:
                in_names.append(name)
        elif alloc.kind == "ExternalOutput":
            shape = tuple(alloc.tensor_shape)
            dtype = mybir.dt.np(alloc.dtype)
            out_names.append(name)
            out_avals.append(jax.core.ShapedArray(shape, dtype))
            zero_outs.append(np.zeros(shape, dtype))
    return partition_name, in_names, out_names, out_avals, zero_outs


def _make_runner(nc):
    import jax
    from jax.experimental.shard_map import shard_map
    from jax.sharding import Mesh, NamedSharding, PartitionSpec
    from concourse import bass2jax

    pname, in_names, out_names, out_avals, zero_outs = _pjrt_parts(nc)
    n_params = len(in_names)
    all_in_names = list(in_names) + list(out_names)
    if pname is not None:
        all_in_names.append(pname)

    def _body(*args):
        operands = list(args)
        if pname is not None:
            operands.append(bass2jax.partition_id_tensor())
        return tuple(
            bass2jax._bass_exec_p.bind(
                *operands,
                out_avals=tuple(out_avals),
                in_names=tuple(all_in_names),
                out_names=tuple(out_names),
                lowering_input_output_aliases=(),
                sim_require_finite=True,
                sim_require_nnan=True,
                nc=nc,
            )
        )

    devices = jax.devices()[:NCORES]
    mesh = Mesh(np.asarray(devices), ("core",))
    spec = PartitionSpec("core")
    n_outs = len(out_names)
    fn = jax.jit(
        shard_map(
            _body,
            mesh=mesh,
            in_specs=(spec,) * (n_params + n_outs),
            out_specs=(spec,) * n_outs,
            check_rep=False,
        ),
        keep_unused=True,
    )
    sh = NamedSharding(mesh, spec)
    return fn, sh, in_names, out_avals, zero_outs


def sim_estimate(nc):
    from concourse.timeline_sim import TimelineSim

    return TimelineSim(nc, no_exec=True).simulate()


def _dev_inputs(x, sh, in_names, zero_outs):
    import jax

    xbe, xbo = _inputs_for(x)
    arrs = {
        "xbe": xbe.reshape(NCORES * UPC, GN),
        "xbo": xbo.reshape(NCORES * UPC, GN),
    }
    dev = [jax.device_put(arrs[nm], sh) for nm in in_names]
    dev += [
        jax.device_put(np.zeros((NCORES * z.shape[0], *z.shape[1:]), z.dtype), sh)
        for z in zero_outs
    ]
    return dev


def bench(x, spatial_kernel, rep_lo=11, rep_hi=41, reps=16):
    """Marginal per-iteration device time via interleaved repeat-NEFF timing."""
    import time
    import jax

    x = np.ascontiguousarray(np.asarray(x, dtype=np.float32))
    sk = np.asarray(spatial_kernel, dtype=np.float64).reshape(-1)
    key = sk.tobytes()
    if key not in _cache:
        _cache[key] = _build(sk)
    nc1 = _cache[key]

    runners = {}
    for n in (rep_lo, rep_hi):
        key_r = (key, n)
        if key_r not in _cache:
            _cache[key_r] = _build(sk, repeat=n)
        fn, sh, in_names, out_avals, zero_outs = _make_runner(_cache[key_r])
        dev_in = _dev_inputs(x, sh, in_names, zero_outs)
        jax.block_until_ready(fn(*dev_in))
        runners[n] = (fn, dev_in)

    fn1, sh1, in_names1, out_avals1, zero_outs1 = _make_runner(nc1)
    dev_in1 = _dev_inputs(x, sh1, in_names1, zero_outs1)
    outs = fn1(*dev_in1)
    jax.block_until_ready(outs)
    outs_np = np.asarray(outs[0]).reshape(NCORES, UPC, C, BLK, BLK)

    samples = {n: [] for n in runners}
    for _ in range(reps):
        for n, (fn, dev_in) in runners.items():
            t0 = time.perf_counter()
            jax.block_until_ready(fn(*dev_in))
            samples[n].append(time.perf_counter() - t0)
    med = {n: float(np.median(np.asarray(t))) for n, t in samples.items()}
    marg_ns = (med[rep_hi] - med[rep_lo]) / (rep_hi - rep_lo) * 1e9
    stats = {
        "chain_ns": marg_ns,
        f"t_r{rep_lo}": med[rep_lo],
        f"t_r{rep_hi}": med[rep_hi],
        "sim_r1_ns": sim_estimate(nc1),
    }
    full = _unshard(outs_np.astype(np.float32))
    return stats, full


def kernel(x, spatial_kernel):
    import jax
    from concourse.bass_utils import run_bass_kernel_spmd

    x = np.ascontiguousarray(np.asarray(x, dtype=np.float32))
    sk = np.asarray(spatial_kernel, dtype=np.float64).reshape(-1)

    key = sk.tobytes()
    if key not in _cache:
        _cache[key] = _build(sk)
    nc = _cache[key]

    rkey = (key, "runner")
    if rkey in _cache:
        fn, sh, in_names, out_avals, zero_outs = _cache[rkey]
        dev_in = _dev_inputs(x, sh, in_names, zero_outs)
        outs = fn(*dev_in)
        jax.block_until_ready(outs)
        out_np = np.asarray(outs[0]).reshape(NCORES, UPC, C, BLK, BLK)
        return _unshard(out_np.astype(np.float32))

    xbe, xbo = _inputs_for(x)
    in_maps = [{"xbe": xbe[c], "xbo": xbo[c]} for c in range(NCORES)]
    res = run_bass_kernel_spmd(nc, in_maps, list(range(NCORES)), trace=TRACE)
    LAST_STATS.clear()
    LAST_STATS.update(
        exec_time_ns=res.exec_time_ns,
        mean_exec_time_ns=res.mean_exec_time_ns,
    )
    _cache[rkey] = _make_runner(nc)
    outs = np.stack([r["out"] for r in res.results]).astype(np.float32)
    return _unshard(outs.reshape(NCORES, UPC, C, BLK, BLK))
